# revision 25
# baseline (speedup 1.0000x reference)
"""Trainium2 Bass kernel for a 16-head causal MHA layer.

Problem: x:[2,2048,1024] f32, wq/wk/wv/wo:[1024,1024] f32 (Linear-style
[out,in] weights), causal softmax attention with 16 heads of dim 64.

Sharding across the 8 NeuronCores: 2-way data parallel over batch x
4-way tensor parallel over heads.  Core c handles batch c//4 and the 4
heads 4*(c%4) .. 4*(c%4)+3 (feature slice of 256 rows of wq/wk/wv and
256 columns of wo).  Each core produces a partial [2048,1024] output
(its 4 heads' contribution, already projected through its wo slice);
the host sums the 4 partials per batch.

Device dataflow (all matmul inputs fp16, fp32 PSUM accumulation):
  - host uploads x already transposed per batch: xT [1024, 2048] fp16,
    and weights pre-arranged partition-major so every weight DMA is a
    contiguous per-partition line
  - while x streams in, a dc-gated projection prologue accumulates
    qT/kT for the first two 512-query chunks, so attention starts as
    soon as the input load finishes
  - qT/kT = W @ xT in [feat, token] layout; v in [token, feat] layout,
    with a constant-1 column appended per head (v|1)
  - scoresT[k,q] = kT_h.T-block @ qT_h (64-dim contraction, two heads
    row-packed via tile_position), exp on ACT straight out of PSUM (no
    max subtraction: |scores/8| < ~4 so exp is safe in fp32/fp16),
    causal mask applied only on diagonal blocks via a 0/1 mask multiply
  - out_unnorm.T | l = (v|1).T-block @ expT accumulated over k blocks
    (the appended ones-column yields the softmax denominator l for free)
  - 1/l via a DRAM-roundtrip transpose to [128,x] + DVE reciprocal,
    broadcast back across partitions with column-packed identity
    matmuls, multiplied into out_unnorm.T
  - y = outT.T @ woT accumulated over the 256-dim feature slice
"""

import numpy as np

S = 2048          # sequence length (one batch per core)
D = 1024          # model dim
HL = 4            # heads handled per core
DH = 64           # head dim
F = HL * DH       # 256 local features
DC = D // 128     # 8 d_model chunks of 128
FC = F // 128     # 2 feature chunks of 128
NT = S // 128     # 16 token tiles
NQ = S // 512     # 4 query chunks of 512

_CACHE = {}


def _build_program(dbg=False):
    key = ("nc", dbg)
    if key in _CACHE:
        return _CACHE[key]

    import concourse.bacc as bacc
    import concourse.bass as bass
    import concourse.mybir as mybir
    import concourse.tile as tile

    f16 = mybir.dt.float16
    f32 = mybir.dt.float32
    Exp = mybir.ActivationFunctionType.Exp

    nc = bacc.Bacc("TRN2", target_bir_lowering=False, debug=False)

    xT_d = nc.dram_tensor("xT", [DC, 128, S], f16, kind="ExternalInput")
    wq_d = nc.dram_tensor("wqT", [128, DC, F], f16, kind="ExternalInput")
    wk_d = nc.dram_tensor("wkT", [128, DC, F], f16, kind="ExternalInput")
    wv_d = nc.dram_tensor("wvT", [128, DC, F], f16, kind="ExternalInput")
    wo_d = nc.dram_tensor("woT", [128, FC, D], f16, kind="ExternalInput")
    mask_d = nc.dram_tensor("mask", [128, 896], f16, kind="ExternalInput")
    ident_d = nc.dram_tensor("ident", [128, 128], f16, kind="ExternalInput")
    y_d = nc.dram_tensor("y", [S, D], f16, kind="ExternalOutput")
    if dbg:
        qT_dbg = nc.dram_tensor("qT_dbg", [128, FC, S], f16, kind="ExternalOutput")
        kT_dbg = nc.dram_tensor("kT_dbg", [128, FC, S], f16, kind="ExternalOutput")
        v_dbg = nc.dram_tensor("v_dbg", [128, NT, HL, DH + 1], f16, kind="ExternalOutput")
        outT_dbg = nc.dram_tensor("outT_dbg", [128, FC, S], f16, kind="ExternalOutput")
        l_dbg = nc.dram_tensor("l_dbg", [HL * S], f32, kind="ExternalOutput")
        lt_dbg = nc.dram_tensor("lt_dbg", [128, HL * NT], f32, kind="ExternalOutput")

    with tile.TileContext(nc) as tc:
        with tc.tile_pool(name="const", bufs=1) as cpool, \
             tc.tile_pool(name="dscr", bufs=1,
                          space=bass.MemorySpace.DRAM) as dpool:
            l_dram = dpool.tile([HL * S], f32)
            xT = cpool.tile([128, DC, S], f16)
            wq = cpool.tile([128, DC, F], f16)
            wk = cpool.tile([128, DC, F], f16)
            wv = cpool.tile([128, DC, F], f16)
            wo = cpool.tile([128, FC, D], f16)
            mask = cpool.tile([128, 896], f16)
            ident = cpool.tile([128, 128], f16)
            qT = cpool.tile([128, FC, S], f16)
            kT = cpool.tile([128, FC, S], f16)
            v = cpool.tile([128, NT, HL, DH + 1], f16)
            outT = cpool.tile([128, FC, S], f16)
            l_row = cpool.tile([1, HL * S], f32)
            lT = cpool.tile([128, HL * NT], f32)
            recipT = cpool.tile([128, HL * NT], f32)
            recipT16 = cpool.tile([128, HL * NT], f16)

            # load order: q/k weights first (the dc-gated projection pass
            # needs them immediately), then x chunks; wv behind x (the v
            # pass runs right after the load); mask/ident late; wo (not
            # needed until the first wo_tile) rides the slow gpsimd
            # software queue
            nc.sync.dma_start(wq[:], wq_d[:])
            nc.scalar.dma_start(wk[:], wk_d[:])
            for dc in range(0, DC, 2):
                nc.sync.dma_start(xT[:, dc, :], xT_d[dc])
                nc.scalar.dma_start(xT[:, dc + 1, :], xT_d[dc + 1])
            nc.sync.dma_start(wv[:], wv_d[:])
            nc.scalar.dma_start(mask[:], mask_d[:])
            nc.scalar.dma_start(ident[:], ident_d[:])
            nc.gpsimd.dma_start(wo[:], wo_d[:])

            # ones columns for the softmax-denominator trick
            nc.gpsimd.memset(v[:, :, :, DH:DH + 1], 1.0)

            # ---- dc-gated projection prologue --------------------------
            # While xT chunks stream in, accumulate qT/kT for query
            # chunks 0 AND 1 chunk by chunk (8 PSUM banks), so attention
            # starts right after the load completes with a quarter of the
            # q/k projection work already done.  Warmup matmuls keep the
            # PE HAM clock-gate at 8/8 before the first chunk lands; they
            # write junk into the q1 accumulator, which the dc=0
            # start=True matmul clears.
            with tc.tile_pool(name="p0_ps", bufs=1,
                              space=bass.MemorySpace.PSUM) as p0p, \
                 tc.tile_pool(name="p0_sb", bufs=1) as p0sb:
                qp = [[p0p.tile([128, 512], f32, name=f"p0_q{t5}{fc}")
                       for fc in range(FC)] for t5 in range(2)]
                kp = [[p0p.tile([128, 512], f32, name=f"p0_k{t5}{fc}")
                       for fc in range(FC)] for t5 in range(2)]
                warm = p0sb.tile([128, 128], f16)
                nc.vector.memset(warm[:], 1.0)
                for _ in range(52):
                    nc.tensor.matmul(
                        qp[1][0][:, 0:256], warm[:],
                        warm[:, 0:1].to_broadcast((128, 256)),
                        start=True, stop=True, skip_group_check=True)

                for dc in range(DC):
                    st = (dc == 0)
                    sp = (dc == DC - 1)
                    for t5 in range(2):
                        for w_sb, p in ((wq, qp[t5]), (wk, kp[t5])):
                            for fc in range(FC):
                                nc.tensor.matmul(
                                    p[fc][:],
                                    w_sb[:, dc, fc * 128:(fc + 1) * 128],
                                    xT[:, dc, t5 * 512:(t5 + 1) * 512],
                                    start=st, stop=sp,
                                    skip_group_check=(t5 == 1 and fc == 0))
                for t5 in range(2):
                    for fc in range(FC):
                        nc.vector.tensor_copy(
                            qT[:, fc, t5 * 512:(t5 + 1) * 512],
                            qp[t5][fc][:])
                        nc.vector.tensor_copy(
                            kT[:, fc, t5 * 512:(t5 + 1) * 512],
                            kp[t5][fc][:])

            # ---- attention + normalize + output projection -------------
            # qc-major: all heads for query-chunk qc, then (lagged by one
            # chunk so every dependency is long ready) the softmax
            # normalization and wo projection for chunk qc-1.  The wo/proj
            # matmuls fill the PE bubbles of the exp-bound attention loop.
            with tc.tile_pool(name="sc_ps", bufs=2,
                              space=bass.MemorySpace.PSUM) as scp, \
                 tc.tile_pool(name="av_ps", bufs=2,
                              space=bass.MemorySpace.PSUM) as avp, \
                 tc.tile_pool(name="ybc_ps", bufs=2,
                              space=bass.MemorySpace.PSUM) as ybcp, \
                 tc.tile_pool(name="p_sb", bufs=6) as ppool, \
                 tc.tile_pool(name="y_sb", bufs=4) as ysb_pool:

                # fillers are GENERATORS yielding every ~2 matmuls, so the
                # scheduler can squeeze fine-grained PE work into the
                # exp-latency windows of the attention inner loop
                def proj_qk_group(w_sb, dstT, fc, t5):
                    ps = ybcp.tile([128, 512], f32, tag="ybc",
                                   name=f"ps_{t5}_{fc}")
                    for dc in range(DC):
                        nc.tensor.matmul(
                            ps[:],
                            w_sb[:, dc, fc * 128:(fc + 1) * 128],
                            xT[:, dc, t5 * 512:(t5 + 1) * 512],
                            start=(dc == 0), stop=(dc == DC - 1))
                        if dc % 2 == 1:
                            yield
                    nc.vector.tensor_copy(
                        dstT[:, fc, t5 * 512:(t5 + 1) * 512], ps[:])

                def proj_v_group(tt):
                    psv = ybcp.tile([128, F], f32, tag="ybc",
                                    name=f"psv_{tt}")
                    for dc in range(DC):
                        nc.tensor.matmul(
                            psv[:],
                            xT[:, dc, tt * 128:(tt + 1) * 128],
                            wv[:, dc, :],
                            start=(dc == 0), stop=(dc == DC - 1))
                        if dc % 4 == 3:
                            yield
                    nc.vector.tensor_copy(
                        v[:, tt, :, 0:DH],
                        psv.rearrange("p (h d) -> p h d", h=HL))

                import collections
                fillers = collections.deque()

                def run_filler(n):
                    done = 0
                    while done < n and fillers:
                        g = fillers[0]
                        try:
                            next(g)
                            done += 1
                        except StopIteration:
                            fillers.popleft()

                def run_all_fillers():
                    while fillers:
                        g = fillers.popleft()
                        for _ in g:
                            pass

                def att_hc(qc, hc):
                    if True:
                        avs = []
                        for hp2 in range(2):
                            av = avp.tile([DH + 1, 512], f32, tag="av",
                                          name=f"av_{hc}_{qc}_{hp2}")
                            avs.append(av)
                        for g in range(qc + 1):
                            diag = (g == qc)
                            for half in range(2):
                                # (offset, width) of each k-block's valid
                                # q-span inside the p tile; diagonal blocks
                                # are clipped to q >= k_block_start
                                if diag:
                                    rs = [2 * half, 2 * half + 1]
                                    spans = [(128 * r, 512 - 128 * r)
                                             for r in rs]
                                else:
                                    spans = [(0, 512), (0, 512)]
                                offs = [0, spans[0][1]]
                                scs = []
                                for hp2 in range(2):
                                    sc = scp.tile([128, 1024], f32, tag="sc",
                                                  name=f"sc_{hc}_{qc}_{g}_{half}_{hp2}")
                                    scs.append(sc)
                                for r2 in range(2):
                                    kb = 4 * g + 2 * half + r2
                                    qo, w = spans[r2]
                                    for hp2 in range(2):
                                        hp = hp2 * 64
                                        nc.tensor.matmul(
                                            scs[hp2][:, offs[r2]:offs[r2] + w],
                                            kT[hp:hp + 64, hc,
                                               kb * 128:(kb + 1) * 128],
                                            qT[hp:hp + 64, hc,
                                               qc * 512 + qo:(qc + 1) * 512],
                                            start=True, stop=True,
                                            tile_position=(hp, 0))
                                width = offs[1] + spans[1][1]
                                p_sbs = []
                                for hp2 in range(2):
                                    p_sb = ppool.tile([128, 1024], f16,
                                                      tag=f"p{hp2}",
                                                      name=f"p_{hc}_{qc}_{g}_{half}_{hp2}")
                                    p_sbs.append(p_sb)
                                    nc.scalar.activation(
                                        p_sb[:, 0:width],
                                        scs[hp2][:, 0:width], Exp)
                                    if diag:
                                        # only the first 128 columns of a
                                        # clipped block straddle the diagonal
                                        for r2 in range(2):
                                            nc.vector.tensor_mul(
                                                p_sb[:, offs[r2]:offs[r2] + 128],
                                                p_sb[:, offs[r2]:offs[r2] + 128],
                                                mask[:, 384:512])
                                # PE filler work lands here, between the exp
                                # issues and the AV matmuls, so the PE array
                                # chews on projections/wo while ACT computes
                                # the exps the AV matmuls are waiting for
                                run_filler(3)
                                for hp2 in range(2):
                                    h = hc * 2 + hp2
                                    p_sb = p_sbs[hp2]
                                    for r2 in range(2):
                                        kb = 4 * g + 2 * half + r2
                                        qo, w = spans[r2]
                                        nc.tensor.matmul(
                                            avs[hp2][:, qo:512],
                                            v[:, kb, h, :],
                                            p_sb[:, offs[r2]:offs[r2] + w],
                                            start=(kb == 0),
                                            stop=(kb == 4 * qc + 3))
                        for hp2 in range(2):
                            h = hc * 2 + hp2
                            hp = hp2 * 64
                            nc.vector.tensor_copy(
                                outT[hp:hp + 64, hc, qc * 512:(qc + 1) * 512],
                                avs[hp2][0:DH, :])
                            # denominators: Vector (no DMA-wait risk; keeps
                            # the exp-bound Scalar queue free)
                            nc.vector.tensor_copy(
                                l_row[0:1, h * S + qc * 512:
                                      h * S + (qc + 1) * 512],
                                avs[hp2][DH:DH + 1, :])
                            seg = slice(h * S + qc * 512,
                                        h * S + (qc + 1) * 512)
                            nc.sync.dma_start(l_dram[seg], l_row[0:1, seg])
                            nc.sync.dma_start(
                                lT[:, h * NT + 4 * qc:h * NT + 4 * qc + 4],
                                l_dram[seg].rearrange("(t p) -> p t", p=128))

                def norm_pair(qc, hc):
                    # 1/l on the [q-partition] transposed copy, broadcast
                    # back over the 64 dh rows with K=128 matmuls against
                    # the identity (no DMA in this chain — a DMA ladder
                    # here serializes the in-order sync queue and stalls
                    # the whole kernel).  The two heads' broadcasts are
                    # column-packed into one [128,512] PSUM tile via
                    # tile_position so their matmuls run concurrently.
                    bc2 = ybcp.tile([128, 512], f32, tag="ybc",
                                    name=f"bc_{hc}_{qc}")
                    for hp2 in range(2):
                        h = hc * 2 + hp2
                        c = slice(h * NT + 4 * qc, h * NT + 4 * qc + 4)
                        nc.vector.reciprocal(recipT[:, c], lT[:, c])
                        nc.vector.tensor_copy(recipT16[:, c], recipT[:, c])
                    for t4 in range(4):
                        for hp2 in range(2):
                            h = hc * 2 + hp2
                            hp = hp2 * 64
                            col = h * NT + 4 * qc + t4
                            nc.tensor.matmul(
                                bc2[hp:hp + 64, t4 * 128:(t4 + 1) * 128],
                                recipT16[:, col:col + 1]
                                .to_broadcast((128, DH)),
                                ident[:],
                                start=True, stop=True,
                                tile_position=(0, hp))
                    yield
                    for hp2 in range(2):
                        hp = hp2 * 64
                        nc.vector.tensor_mul(
                            outT[hp:hp + 64, hc, qc * 512:(qc + 1) * 512],
                            outT[hp:hp + 64, hc, qc * 512:(qc + 1) * 512],
                            bc2[hp:hp + 64, :])

                def wo_tile(qt, oc):
                    yps = ybcp.tile([128, 512], f32, tag="ybc",
                                    name=f"yps_{qt}_{oc}")
                    for fc in range(FC):
                        nc.tensor.matmul(
                            yps[:],
                            outT[:, fc, qt * 128:(qt + 1) * 128],
                            wo[:, fc, oc * 512:(oc + 1) * 512],
                            start=(fc == 0), stop=(fc == FC - 1))
                    yield
                    ysb = ysb_pool.tile([128, 512], f16, tag="ysb",
                                        name=f"ysb_{qt}_{oc}")
                    nc.vector.tensor_copy(ysb[:], yps[:])
                    nc.sync.dma_start(
                        y_d[qt * 128:(qt + 1) * 128,
                            oc * 512:(oc + 1) * 512],
                        ysb[:])

                # v for key blocks 0/1 inline (the very first AV matmuls
                # need them); 2/3 as lead fillers during attention chunk 0
                for tt in range(2):
                    for _ in proj_v_group(tt):
                        pass
                for tt in range(2, 4):
                    fillers.append(proj_v_group(tt))
                for qc in range(NQ):
                    if qc + 1 < NQ:
                        if qc + 1 >= 2:  # t5=1 was done in the prologue
                            for w_sb, dstT in ((wq, qT), (wk, kT)):
                                for fc in range(FC):
                                    fillers.append(
                                        proj_qk_group(w_sb, dstT, fc,
                                                      qc + 1))
                        for tt in range(4 * (qc + 1), 4 * (qc + 2)):
                            fillers.append(proj_v_group(tt))
                    if qc >= 1:
                        for hcx in range(FC):
                            fillers.append(norm_pair(qc - 1, hcx))
                        if qc < NQ - 1:
                            for qt in range(4 * (qc - 1), 4 * qc):
                                for oc in range(2):
                                    fillers.append(wo_tile(qt, oc))
                    att_hc(qc, 0)
                    if qc == NQ - 1:
                        # last chunk: normalize head-pair 0 while head-pair
                        # 1's attention still runs, hiding the l transpose
                        # roundtrip of the tail
                        fillers.append(norm_pair(qc, 0))
                    att_hc(qc, 1)
                    run_all_fillers()
                # chunk-2 wo tiles were held out of the filler pool so
                # their matmuls cover the last denominator-transpose DMA
                # roundtrip instead of leaving the PE idle
                for qt in range(4 * (NQ - 2), 4 * (NQ - 1)):
                    for oc in range(2):
                        for g in wo_tile(qt, oc):
                            pass
                for g in norm_pair(NQ - 1, 1):
                    pass
                for qt in range(4 * (NQ - 1), 4 * NQ):
                    for oc in range(2):
                        for g in wo_tile(qt, oc):
                            pass

            if dbg:
                nc.sync.dma_start(qT_dbg[:], qT[:])
                nc.sync.dma_start(kT_dbg[:], kT[:])
                nc.sync.dma_start(v_dbg[:], v[:])
                nc.sync.dma_start(outT_dbg[:], outT[:])
                nc.sync.dma_start(l_dbg[:], l_row[0:1, :])
                nc.sync.dma_start(lt_dbg[:], lT[:])

    nc.compile()

    from concourse.bass_interp import get_hw_module
    nc.m = get_hw_module(nc.m)

    _CACHE[key] = nc
    return nc


def _make_mask():
    # mask[p, j] = 1 where (j - p) >= 384; slices of width 512 at offset
    # 384-128*r give the causal mask for a diagonal block at relative
    # position r (k block kb = 4*qc + r vs the 512-wide q chunk qc)
    j = np.arange(896)[None, :]
    p = np.arange(128)[:, None]
    return ((j - p) >= 384).astype(np.float16)


def kernel(x, wq, wk, wv, wo):
    x = np.asarray(x, dtype=np.float32)
    wq = np.asarray(wq, dtype=np.float32)
    wk = np.asarray(wk, dtype=np.float32)
    wv = np.asarray(wv, dtype=np.float32)
    wo = np.asarray(wo, dtype=np.float32)

    from concourse import bass_utils

    nc = _build_program()
    mask = _make_mask()

    def part_major(wT, nchunk, width):
        # [d, f] -> [128, nchunk, width] with d = chunk*128 + partition
        return np.ascontiguousarray(
            wT.reshape(nchunk, 128, width).transpose(1, 0, 2))

    in_maps = []
    for c in range(8):
        b = c // 4
        hg = c % 4
        fs = slice(hg * F, (hg + 1) * F)
        xT = np.ascontiguousarray(x[b].T).astype(np.float16).reshape(DC, 128, S)
        wqT = np.ascontiguousarray((wq[fs, :] * 0.125).T).astype(np.float16)
        wkT = np.ascontiguousarray(wk[fs, :].T).astype(np.float16)
        wvT = np.ascontiguousarray(wv[fs, :].T).astype(np.float16)
        woT = np.ascontiguousarray(wo[:, fs].T).astype(np.float16)
        in_maps.append({
            "xT": xT,
            "wqT": part_major(wqT, DC, F),
            "wkT": part_major(wkT, DC, F),
            "wvT": part_major(wvT, DC, F),
            "woT": part_major(woT, FC, D),
            "mask": mask,
            "ident": np.eye(128, dtype=np.float16),
        })

    res = bass_utils.run_bass_kernel_spmd(nc, in_maps, core_ids=list(range(8)))
    ys = [res.results[c]["y"].astype(np.float32) for c in range(8)]
    out = np.stack([ys[0] + ys[1] + ys[2] + ys[3],
                    ys[4] + ys[5] + ys[6] + ys[7]])
    return out


# revision 27
# speedup vs baseline: 1.0547x; 1.0547x over previous
"""Trainium2 Bass kernel for a 16-head causal MHA layer.

Problem: x:[2,2048,1024] f32, wq/wk/wv/wo:[1024,1024] f32 (Linear-style
[out,in] weights), causal softmax attention with 16 heads of dim 64.

Sharding across the 8 NeuronCores: 2-way data parallel over batch x
4-way tensor parallel over heads.  Core c handles batch c//4 and the 4
heads 4*(c%4) .. 4*(c%4)+3 (feature slice of 256 rows of wq/wk/wv and
256 columns of wo).  Each core produces a partial [2048,1024] output
(its 4 heads' contribution, already projected through its wo slice);
the host sums the 4 partials per batch.

Device dataflow (all matmul inputs fp16, fp32 PSUM accumulation):
  - host uploads x already transposed per batch: xT [1024, 2048] fp16,
    and weights pre-arranged partition-major so every weight DMA is a
    contiguous per-partition line
  - while x streams in, a dc-gated projection prologue accumulates
    qT/kT for the first two 512-query chunks, so attention starts as
    soon as the input load finishes
  - qT/kT = W @ xT in [feat, token] layout; v in [token, feat] layout,
    with a constant-1 column appended per head (v|1)
  - scoresT[k,q] = kT_h.T-block @ qT_h (64-dim contraction, two heads
    row-packed via tile_position), exp on ACT straight out of PSUM (no
    max subtraction: |scores/8| < ~4 so exp is safe in fp32/fp16),
    causal mask applied only on diagonal blocks via a 0/1 mask multiply
  - out_unnorm.T | l = (v|1).T-block @ expT accumulated over k blocks
    (the appended ones-column yields the softmax denominator l for free)
  - 1/l via a DRAM-roundtrip transpose to [128,x] + DVE reciprocal,
    broadcast back across partitions with column-packed identity
    matmuls, multiplied into out_unnorm.T
  - y = outT.T @ woT accumulated over the 256-dim feature slice
"""

import numpy as np

S = 2048          # sequence length (one batch per core)
D = 1024          # model dim
HL = 4            # heads handled per core
DH = 64           # head dim
F = HL * DH       # 256 local features
DC = D // 128     # 8 d_model chunks of 128
FC = F // 128     # 2 feature chunks of 128
NT = S // 128     # 16 token tiles
NQ = S // 512     # 4 query chunks of 512

_CACHE = {}


def _build_program(dbg=False):
    key = ("nc", dbg)
    if key in _CACHE:
        return _CACHE[key]

    import concourse.bacc as bacc
    import concourse.bass as bass
    import concourse.mybir as mybir
    import concourse.tile as tile

    f16 = mybir.dt.float16
    f32 = mybir.dt.float32
    Exp = mybir.ActivationFunctionType.Exp

    nc = bacc.Bacc("TRN2", target_bir_lowering=False, debug=False)

    xT_d = nc.dram_tensor("xT", [DC, 128, S], f16, kind="ExternalInput")
    wq_d = nc.dram_tensor("wqT", [128, DC, F], f16, kind="ExternalInput")
    wk_d = nc.dram_tensor("wkT", [128, DC, F], f16, kind="ExternalInput")
    wv_d = nc.dram_tensor("wvT", [128, DC, F], f16, kind="ExternalInput")
    wo_d = nc.dram_tensor("woT", [128, FC, D], f16, kind="ExternalInput")
    mask_d = nc.dram_tensor("mask", [128, 896], f16, kind="ExternalInput")
    ident_d = nc.dram_tensor("ident", [128, 128], f16, kind="ExternalInput")
    y_d = nc.dram_tensor("y", [S, D], f16, kind="ExternalOutput")
    if dbg:
        qT_dbg = nc.dram_tensor("qT_dbg", [128, FC, S], f16, kind="ExternalOutput")
        kT_dbg = nc.dram_tensor("kT_dbg", [128, FC, S], f16, kind="ExternalOutput")
        v_dbg = nc.dram_tensor("v_dbg", [128, NT, HL, DH + 1], f16, kind="ExternalOutput")
        outT_dbg = nc.dram_tensor("outT_dbg", [128, FC, S], f16, kind="ExternalOutput")
        l_dbg = nc.dram_tensor("l_dbg", [HL * S], f32, kind="ExternalOutput")
        lt_dbg = nc.dram_tensor("lt_dbg", [128, HL * NT], f32, kind="ExternalOutput")

    with tile.TileContext(nc) as tc:
        with tc.tile_pool(name="const", bufs=1) as cpool, \
             tc.tile_pool(name="dscr", bufs=1,
                          space=bass.MemorySpace.DRAM) as dpool:
            l_dram = dpool.tile([HL * S], f32)
            xT = cpool.tile([128, DC, S], f16)
            wq = cpool.tile([128, DC, F], f16)
            wk = cpool.tile([128, DC, F], f16)
            wv = cpool.tile([128, DC, F], f16)
            wo = cpool.tile([128, FC, D], f16)
            mask = cpool.tile([128, 896], f16)
            ident = cpool.tile([128, 128], f16)
            qT = cpool.tile([128, FC, S], f16)
            kT = cpool.tile([128, FC, S], f16)
            v = cpool.tile([128, NT, HL, DH + 1], f16)
            outT = cpool.tile([128, FC, S], f16)
            l_row = cpool.tile([1, HL * S], f32)
            lT = cpool.tile([128, HL * NT], f32)
            recipT = cpool.tile([128, HL * NT], f32)
            recipT16 = cpool.tile([128, HL * NT], f16)

            # load order: q/k weights first (the dc-gated projection pass
            # needs them immediately), then x chunks; wv behind x (the v
            # pass runs right after the load); mask/ident late; wo (not
            # needed until the first wo_tile) rides the slow gpsimd
            # software queue
            nc.sync.dma_start(wq[:], wq_d[:])
            nc.scalar.dma_start(wk[:], wk_d[:])
            for dc in range(0, DC, 2):
                nc.sync.dma_start(xT[:, dc, :], xT_d[dc])
                nc.scalar.dma_start(xT[:, dc + 1, :], xT_d[dc + 1])
            nc.sync.dma_start(wv[:], wv_d[:])
            nc.scalar.dma_start(mask[:], mask_d[:])
            nc.scalar.dma_start(ident[:], ident_d[:])
            nc.gpsimd.dma_start(wo[:], wo_d[:])

            # ones columns for the softmax-denominator trick
            nc.gpsimd.memset(v[:, :, :, DH:DH + 1], 1.0)

            # ---- dc-gated projection prologue --------------------------
            # While xT chunks stream in, accumulate qT/kT for query
            # chunks 0 AND 1 chunk by chunk (8 PSUM banks), so attention
            # starts right after the load completes with a quarter of the
            # q/k projection work already done.  Warmup matmuls keep the
            # PE HAM clock-gate at 8/8 before the first chunk lands; they
            # write junk into the q1 accumulator, which the dc=0
            # start=True matmul clears.
            with tc.tile_pool(name="p0_ps", bufs=1,
                              space=bass.MemorySpace.PSUM) as p0p, \
                 tc.tile_pool(name="p0_sb", bufs=1) as p0sb:
                qp = [[p0p.tile([128, 512], f32, name=f"p0_q{t5}{fc}")
                       for fc in range(FC)] for t5 in range(2)]
                kp = [[p0p.tile([128, 512], f32, name=f"p0_k{t5}{fc}")
                       for fc in range(FC)] for t5 in range(2)]
                warm = p0sb.tile([128, 128], f16)
                nc.vector.memset(warm[:], 1.0)
                for _ in range(52):
                    nc.tensor.matmul(
                        qp[1][0][:, 0:256], warm[:],
                        warm[:, 0:1].to_broadcast((128, 256)),
                        start=True, stop=True, skip_group_check=True)

                for dc in range(DC):
                    st = (dc == 0)
                    sp = (dc == DC - 1)
                    for t5 in range(2):
                        for w_sb, p in ((wq, qp[t5]), (wk, kp[t5])):
                            for fc in range(FC):
                                nc.tensor.matmul(
                                    p[fc][:],
                                    w_sb[:, dc, fc * 128:(fc + 1) * 128],
                                    xT[:, dc, t5 * 512:(t5 + 1) * 512],
                                    start=st, stop=sp,
                                    skip_group_check=(t5 == 1 and fc == 0))
                for t5 in range(2):
                    for fc in range(FC):
                        nc.vector.tensor_copy(
                            qT[:, fc, t5 * 512:(t5 + 1) * 512],
                            qp[t5][fc][:])
                        nc.vector.tensor_copy(
                            kT[:, fc, t5 * 512:(t5 + 1) * 512],
                            kp[t5][fc][:])

            # ---- attention + normalize + output projection -------------
            # qc-major: all heads for query-chunk qc, then (lagged by one
            # chunk so every dependency is long ready) the softmax
            # normalization and wo projection for chunk qc-1.  The wo/proj
            # matmuls fill the PE bubbles of the exp-bound attention loop.
            with tc.tile_pool(name="sc_ps", bufs=2,
                              space=bass.MemorySpace.PSUM) as scp, \
                 tc.tile_pool(name="av_ps", bufs=2,
                              space=bass.MemorySpace.PSUM) as avp, \
                 tc.tile_pool(name="ybc_ps", bufs=2,
                              space=bass.MemorySpace.PSUM) as ybcp, \
                 tc.tile_pool(name="p_sb", bufs=6) as ppool, \
                 tc.tile_pool(name="y_sb", bufs=4) as ysb_pool:

                # fillers are GENERATORS yielding every ~2 matmuls, so the
                # scheduler can squeeze fine-grained PE work into the
                # exp-latency windows of the attention inner loop
                def proj_qk_group(w_sb, dstT, fc, t5):
                    ps = ybcp.tile([128, 512], f32, tag="ybc",
                                   name=f"ps_{t5}_{fc}")
                    for dc in range(DC):
                        nc.tensor.matmul(
                            ps[:],
                            w_sb[:, dc, fc * 128:(fc + 1) * 128],
                            xT[:, dc, t5 * 512:(t5 + 1) * 512],
                            start=(dc == 0), stop=(dc == DC - 1))
                        if dc % 2 == 1:
                            yield
                    nc.vector.tensor_copy(
                        dstT[:, fc, t5 * 512:(t5 + 1) * 512], ps[:])

                def proj_v_group(tt):
                    psv = ybcp.tile([128, F], f32, tag="ybc",
                                    name=f"psv_{tt}")
                    for dc in range(DC):
                        nc.tensor.matmul(
                            psv[:],
                            xT[:, dc, tt * 128:(tt + 1) * 128],
                            wv[:, dc, :],
                            start=(dc == 0), stop=(dc == DC - 1))
                        if dc % 4 == 3:
                            yield
                    nc.vector.tensor_copy(
                        v[:, tt, :, 0:DH],
                        psv.rearrange("p (h d) -> p h d", h=HL))

                import collections
                fillers = collections.deque()

                def run_filler(n):
                    done = 0
                    while done < n and fillers:
                        g = fillers[0]
                        try:
                            next(g)
                            done += 1
                        except StopIteration:
                            fillers.popleft()

                def run_all_fillers():
                    while fillers:
                        g = fillers.popleft()
                        for _ in g:
                            pass

                def att_hc(qc, hc):
                    # early chunks: PE-bound, push filler work hard;
                    # late chunks: ACT-bound, keep the scores->exp chain
                    # tight and inject less between exp and AV
                    depth = 4 if qc <= 1 else 2
                    if True:
                        avs = []
                        for hp2 in range(2):
                            av = avp.tile([DH + 1, 512], f32, tag="av",
                                          name=f"av_{hc}_{qc}_{hp2}")
                            avs.append(av)
                        for g in range(qc + 1):
                            diag = (g == qc)
                            for half in range(2):
                                # (offset, width) of each k-block's valid
                                # q-span inside the p tile; diagonal blocks
                                # are clipped to q >= k_block_start
                                if diag:
                                    rs = [2 * half, 2 * half + 1]
                                    spans = [(128 * r, 512 - 128 * r)
                                             for r in rs]
                                else:
                                    spans = [(0, 512), (0, 512)]
                                offs = [0, spans[0][1]]
                                scs = []
                                for hp2 in range(2):
                                    sc = scp.tile([128, 1024], f32, tag="sc",
                                                  name=f"sc_{hc}_{qc}_{g}_{half}_{hp2}")
                                    scs.append(sc)
                                for r2 in range(2):
                                    kb = 4 * g + 2 * half + r2
                                    qo, w = spans[r2]
                                    for hp2 in range(2):
                                        hp = hp2 * 64
                                        nc.tensor.matmul(
                                            scs[hp2][:, offs[r2]:offs[r2] + w],
                                            kT[hp:hp + 64, hc,
                                               kb * 128:(kb + 1) * 128],
                                            qT[hp:hp + 64, hc,
                                               qc * 512 + qo:(qc + 1) * 512],
                                            start=True, stop=True,
                                            tile_position=(hp, 0))
                                width = offs[1] + spans[1][1]
                                p_sbs = []
                                for hp2 in range(2):
                                    p_sb = ppool.tile([128, 1024], f16,
                                                      tag=f"p{hp2}",
                                                      name=f"p_{hc}_{qc}_{g}_{half}_{hp2}")
                                    p_sbs.append(p_sb)
                                    nc.scalar.activation(
                                        p_sb[:, 0:width],
                                        scs[hp2][:, 0:width], Exp)
                                    if diag:
                                        # only the first 128 columns of a
                                        # clipped block straddle the diagonal
                                        for r2 in range(2):
                                            nc.vector.tensor_mul(
                                                p_sb[:, offs[r2]:offs[r2] + 128],
                                                p_sb[:, offs[r2]:offs[r2] + 128],
                                                mask[:, 384:512])
                                # PE filler work lands here, between the exp
                                # issues and the AV matmuls, so the PE array
                                # chews on projections/wo while ACT computes
                                # the exps the AV matmuls are waiting for
                                run_filler(depth)
                                for hp2 in range(2):
                                    h = hc * 2 + hp2
                                    p_sb = p_sbs[hp2]
                                    for r2 in range(2):
                                        kb = 4 * g + 2 * half + r2
                                        qo, w = spans[r2]
                                        nc.tensor.matmul(
                                            avs[hp2][:, qo:512],
                                            v[:, kb, h, :],
                                            p_sb[:, offs[r2]:offs[r2] + w],
                                            start=(kb == 0),
                                            stop=(kb == 4 * qc + 3))
                        for hp2 in range(2):
                            h = hc * 2 + hp2
                            hp = hp2 * 64
                            nc.vector.tensor_copy(
                                outT[hp:hp + 64, hc, qc * 512:(qc + 1) * 512],
                                avs[hp2][0:DH, :])
                            # denominators: Vector (no DMA-wait risk; keeps
                            # the exp-bound Scalar queue free)
                            nc.vector.tensor_copy(
                                l_row[0:1, h * S + qc * 512:
                                      h * S + (qc + 1) * 512],
                                avs[hp2][DH:DH + 1, :])
                            seg = slice(h * S + qc * 512,
                                        h * S + (qc + 1) * 512)
                            nc.sync.dma_start(l_dram[seg], l_row[0:1, seg])
                            nc.sync.dma_start(
                                lT[:, h * NT + 4 * qc:h * NT + 4 * qc + 4],
                                l_dram[seg].rearrange("(t p) -> p t", p=128))

                def norm_pair(qc, hc):
                    # 1/l on the [q-partition] transposed copy, broadcast
                    # back over the 64 dh rows with K=128 matmuls against
                    # the identity (no DMA in this chain — a DMA ladder
                    # here serializes the in-order sync queue and stalls
                    # the whole kernel).  The two heads' broadcasts are
                    # column-packed into one [128,512] PSUM tile via
                    # tile_position so their matmuls run concurrently.
                    bc2 = ybcp.tile([128, 512], f32, tag="ybc",
                                    name=f"bc_{hc}_{qc}")
                    for hp2 in range(2):
                        h = hc * 2 + hp2
                        c = slice(h * NT + 4 * qc, h * NT + 4 * qc + 4)
                        nc.vector.reciprocal(recipT[:, c], lT[:, c])
                        nc.vector.tensor_copy(recipT16[:, c], recipT[:, c])
                    for t4 in range(4):
                        for hp2 in range(2):
                            h = hc * 2 + hp2
                            hp = hp2 * 64
                            col = h * NT + 4 * qc + t4
                            nc.tensor.matmul(
                                bc2[hp:hp + 64, t4 * 128:(t4 + 1) * 128],
                                recipT16[:, col:col + 1]
                                .to_broadcast((128, DH)),
                                ident[:],
                                start=True, stop=True,
                                tile_position=(0, hp))
                    yield
                    for hp2 in range(2):
                        hp = hp2 * 64
                        nc.vector.tensor_mul(
                            outT[hp:hp + 64, hc, qc * 512:(qc + 1) * 512],
                            outT[hp:hp + 64, hc, qc * 512:(qc + 1) * 512],
                            bc2[hp:hp + 64, :])

                def wo_tile(qt, oc):
                    yps = ybcp.tile([128, 512], f32, tag="ybc",
                                    name=f"yps_{qt}_{oc}")
                    for fc in range(FC):
                        nc.tensor.matmul(
                            yps[:],
                            outT[:, fc, qt * 128:(qt + 1) * 128],
                            wo[:, fc, oc * 512:(oc + 1) * 512],
                            start=(fc == 0), stop=(fc == FC - 1))
                    yield
                    ysb = ysb_pool.tile([128, 512], f16, tag="ysb",
                                        name=f"ysb_{qt}_{oc}")
                    nc.vector.tensor_copy(ysb[:], yps[:])
                    nc.sync.dma_start(
                        y_d[qt * 128:(qt + 1) * 128,
                            oc * 512:(oc + 1) * 512],
                        ysb[:])

                # v for key blocks 0/1 inline (the very first AV matmuls
                # need them); 2/3 as lead fillers during attention chunk 0
                for tt in range(2):
                    for _ in proj_v_group(tt):
                        pass
                for tt in range(2, 4):
                    fillers.append(proj_v_group(tt))
                for qc in range(NQ):
                    if qc + 1 < NQ:
                        if qc + 1 >= 2:  # t5=1 was done in the prologue
                            for w_sb, dstT in ((wq, qT), (wk, kT)):
                                for fc in range(FC):
                                    fillers.append(
                                        proj_qk_group(w_sb, dstT, fc,
                                                      qc + 1))
                        for tt in range(4 * (qc + 1), 4 * (qc + 2)):
                            fillers.append(proj_v_group(tt))
                    if qc >= 1:
                        for hcx in range(FC):
                            fillers.append(norm_pair(qc - 1, hcx))
                        for qt in range(4 * (qc - 1), 4 * qc):
                            for oc in range(2):
                                fillers.append(wo_tile(qt, oc))
                    att_hc(qc, 0)
                    if qc == NQ - 1:
                        # last chunk: normalize head-pair 0 while head-pair
                        # 1's attention still runs, hiding the l transpose
                        # roundtrip of the tail
                        fillers.append(norm_pair(qc, 0))
                    att_hc(qc, 1)
                    run_all_fillers()
                for g in norm_pair(NQ - 1, 1):
                    pass
                for qt in range(4 * (NQ - 1), 4 * NQ):
                    for oc in range(2):
                        for g in wo_tile(qt, oc):
                            pass

            if dbg:
                nc.sync.dma_start(qT_dbg[:], qT[:])
                nc.sync.dma_start(kT_dbg[:], kT[:])
                nc.sync.dma_start(v_dbg[:], v[:])
                nc.sync.dma_start(outT_dbg[:], outT[:])
                nc.sync.dma_start(l_dbg[:], l_row[0:1, :])
                nc.sync.dma_start(lt_dbg[:], lT[:])

    nc.compile()

    from concourse.bass_interp import get_hw_module
    nc.m = get_hw_module(nc.m)

    _CACHE[key] = nc
    return nc


def _make_mask():
    # mask[p, j] = 1 where (j - p) >= 384; slices of width 512 at offset
    # 384-128*r give the causal mask for a diagonal block at relative
    # position r (k block kb = 4*qc + r vs the 512-wide q chunk qc)
    j = np.arange(896)[None, :]
    p = np.arange(128)[:, None]
    return ((j - p) >= 384).astype(np.float16)


def kernel(x, wq, wk, wv, wo):
    x = np.asarray(x, dtype=np.float32)
    wq = np.asarray(wq, dtype=np.float32)
    wk = np.asarray(wk, dtype=np.float32)
    wv = np.asarray(wv, dtype=np.float32)
    wo = np.asarray(wo, dtype=np.float32)

    from concourse import bass_utils

    nc = _build_program()
    mask = _make_mask()

    def part_major(wT, nchunk, width):
        # [d, f] -> [128, nchunk, width] with d = chunk*128 + partition
        return np.ascontiguousarray(
            wT.reshape(nchunk, 128, width).transpose(1, 0, 2))

    in_maps = []
    for c in range(8):
        b = c // 4
        hg = c % 4
        fs = slice(hg * F, (hg + 1) * F)
        xT = np.ascontiguousarray(x[b].T).astype(np.float16).reshape(DC, 128, S)
        wqT = np.ascontiguousarray((wq[fs, :] * 0.125).T).astype(np.float16)
        wkT = np.ascontiguousarray(wk[fs, :].T).astype(np.float16)
        wvT = np.ascontiguousarray(wv[fs, :].T).astype(np.float16)
        woT = np.ascontiguousarray(wo[:, fs].T).astype(np.float16)
        in_maps.append({
            "xT": xT,
            "wqT": part_major(wqT, DC, F),
            "wkT": part_major(wkT, DC, F),
            "wvT": part_major(wvT, DC, F),
            "woT": part_major(woT, FC, D),
            "mask": mask,
            "ident": np.eye(128, dtype=np.float16),
        })

    res = bass_utils.run_bass_kernel_spmd(nc, in_maps, core_ids=list(range(8)))
    ys = [res.results[c]["y"].astype(np.float32) for c in range(8)]
    out = np.stack([ys[0] + ys[1] + ys[2] + ys[3],
                    ys[4] + ys[5] + ys[6] + ys[7]])
    return out
